# revision 11
# baseline (speedup 1.0000x reference)
"""DiT-style transformer block (adaLN + attention + MLP) on 8 Trainium2 cores.

Strategy: pure data-parallel over batch (B=8 == n_cores, one sequence per
core, weights replicated, no collectives). Per core the block is computed
with bf16 matmul operands / fp32 accumulation:

  - conditioning: cm = silu(c) @ w_cond computed transposed (weights
    stationary) so the six [H] vectors land feature-major as per-partition
    scalars.
  - LN + modulate: token-major bn_stats, center+rstd on DVE, PE-transpose to
    feature-major, modulate fused into the PSUM eviction.
  - QKV: k^T,q^T feature-major (weights stationary); v token-major
    (activations stationary) with a ones-column appended per head.
  - attention: scores^T = k^T.T @ q^T per head (K=64), exp on ScalarE with
    the 1/sqrt(dh) folded into the activation scale (no max subtraction:
    |scores/8| < ~1), AV uses [v|1] as stationary so the softmax denominator
    falls out as psum row 64; normalization by broadcasting 1/D.
  - attn out / MLP: grouped accumulation keeping PSUM within 8 banks.
  - the reference's scalar "info" statistics are shipped as per-core
    partial reductions and finished on the host.
"""
import sys

if "/opt/trn_rl_repo" not in sys.path:
    sys.path.insert(0, "/opt/trn_rl_repo")

import numpy as np
import ml_dtypes

B, S, H, NH, DH = 8, 1024, 1024, 16, 64
MLP = 4096
EPS = 1e-6
P = 128
NCORES = 8
ST = S // P      # 8 token tiles
FT = H // P      # 8 feature tiles
MT = MLP // P    # 32 mlp tiles

_CACHE = {}


def _build_nc():
    from contextlib import ExitStack
    import concourse.tile as tile
    from concourse import bacc, mybir
    from concourse.masks import make_identity

    f32, bf16 = mybir.dt.float32, mybir.dt.bfloat16
    Alu = mybir.AluOpType
    Act = mybir.ActivationFunctionType

    nc = bacc.Bacc("TRN2", target_bir_lowering=False, debug=False,
                   num_devices=NCORES)

    x_in = nc.dram_tensor("x", [S, H], f32, kind="ExternalInput").ap()
    c_in = nc.dram_tensor("cvec", [H], f32, kind="ExternalInput").ap()
    wc_in = nc.dram_tensor("wc", [H, 6 * H], bf16, kind="ExternalInput").ap()
    wqkv_in = nc.dram_tensor("wqkv", [H, 3 * H], bf16, kind="ExternalInput").ap()
    wat_in = nc.dram_tensor("wat", [H, H], bf16, kind="ExternalInput").ap()
    wm1_in = nc.dram_tensor("wm1", [H, MLP], bf16, kind="ExternalInput").ap()
    wm2_in = nc.dram_tensor("wm2", [MLP, H], bf16, kind="ExternalInput").ap()

    xout = nc.dram_tensor("xout", [S, H], f32, kind="ExternalOutput").ap()
    # stat partials: cnt[128,64] ssqh[128,64] ssqa[128,8] ssqm[128,8]
    #                ln2m/v[128,16] maxp[128,128]
    st_out = nc.dram_tensor("stats", [P, 288], f32, kind="ExternalOutput").ap()
    recd_out = nc.dram_tensor("recd", [P, 128], f32, kind="ExternalOutput").ap()

    with tile.TileContext(nc) as tc:
        root = ExitStack()
        pool = lambda st, name, bufs, **kw: st.enter_context(
            tc.tile_pool(name=name, bufs=bufs, **kw))
        pp = pool(root, "persist", 1)
        junkp = pool(root, "junk", 2)
        smallp = pool(root, "small", 8)

        ident_b = pp.tile([P, P], bf16, tag="identb")
        make_identity(nc, ident_b[:, :])
        ident_f = pp.tile([P, P], f32, tag="identf")
        make_identity(nc, ident_f[:, :])
        eps_t = pp.tile([P, 1], f32, tag="eps")
        nc.vector.memset(eps_t[:, :], EPS)
        cm_sb = pp.tile([P, 48], f32, tag="cm")
        gate_msa_b = pp.tile([P, H], f32, tag="gmsab")
        gate_mlp_b = pp.tile([P, H], f32, tag="gmlpb")
        st_cnt = pp.tile([P, 64], f32, tag="stcnt")
        st_ssqh = pp.tile([P, 64], f32, tag="stssqh")
        st_ssqa = pp.tile([P, 8], f32, tag="stssqa")
        st_ssqm = pp.tile([P, 8], f32, tag="stssqm")
        st_ln2 = pp.tile([P, 16], f32, tag="stln2")
        st_maxp = pp.tile([P, 128], f32, tag="stmaxp")
        st_recd = pp.tile([P, 128], f32, tag="strecd")

        # ---------------- P0: conditioning  cm = silu(c) @ w_cond ---------
        ph0 = ExitStack()
        wcp = pool(ph0, "wcpool", 6)
        cmpsp = pool(ph0, "cmps", 4, space="PSUM")
        gtpsp = pool(ph0, "gtps", 2, space="PSUM")
        c_sb = smallp.tile([P, FT], f32, tag="csb")
        nc.sync.dma_start(out=c_sb[:, :],
                          in_=c_in.rearrange("(k p) -> p k", p=P))
        sg = smallp.tile([P, FT], f32, tag="sg")
        nc.scalar.activation(out=sg[:, :], in_=c_sb[:, :], func=Act.Sigmoid)
        c_bf = smallp.tile([P, FT], bf16, tag="cbf")
        nc.vector.tensor_tensor(out=c_bf[:, :], in0=c_sb[:, :], in1=sg[:, :],
                                op=Alu.mult)

        for m in range(48):
            cm_ps = cmpsp.tile([P, 1], f32, tag="cmps", name="cmps")
            for kc in range(FT):
                wcb = wcp.tile([P, P], bf16, tag="wcb", name="wcb")
                nc.sync.dma_start(out=wcb[:, :],
                                  in_=wc_in[kc * P:(kc + 1) * P,
                                            m * P:(m + 1) * P])
                nc.tensor.matmul(cm_ps[:, :], lhsT=wcb[:, :],
                                 rhs=c_bf[:, kc:kc + 1],
                                 start=(kc == 0), stop=(kc == FT - 1))
            nc.scalar.copy(out=cm_sb[:, m:m + 1], in_=cm_ps[:, :])

        for (cols, gb) in ((range(16, 24), gate_msa_b),
                           (range(40, 48), gate_mlp_b)):
            for fi, col in enumerate(cols):
                gt_ps = gtpsp.tile([1, P], f32, tag="gtps")
                nc.tensor.transpose(gt_ps[:, :], cm_sb[:, col:col + 1],
                                    ident_f[:, :])
                grow = smallp.tile([1, P], f32, tag="grow")
                nc.scalar.copy(out=grow[:, :], in_=gt_ps[:, :])
                nc.gpsimd.partition_broadcast(
                    out_ap=gb[:, fi * P:(fi + 1) * P],
                    in_ap=grow[:, :], channels=P)
        ph0.close()

        # ------- long-lived pools, strict LIFO stack order ---------------
        late = ExitStack()       # x1: dies after P7
        x1p = pool(late, "x1p", 1)
        x1 = [x1p.tile([P, H], f32, tag=f"x1_{j}", name=f"x1_{j}") for j in range(ST)]
        yst = ExitStack()        # yT_sb: dies after P4
        yTsbp = pool(yst, "yTsbp", 1)
        yT_sb = [yTsbp.tile([P, S], bf16, tag=f"yT_{t}", name=f"yT_{t}") for t in range(FT)]
        att = ExitStack()        # kqT/v_aug/pT/y_sb/m8: die after P3
        m8p = pool(att, "m8p", 2)
        ysbp = pool(att, "ysbp", 1)
        y_sb = [ysbp.tile([P, S], bf16, tag=f"y_{j}", name=f"y_{j}")
                for j in range(ST)]
        kqTp = pool(att, "kqTp", 1)
        vaugp = pool(att, "vaugp", 1)
        pTp = pool(att, "pTp", 10)
        kqT = [kqTp.tile([P, S], bf16, tag=f"kqT_{m}", name=f"kqT_{m}") for m in range(16)]
        v_aug = [vaugp.tile([P, NH, 65], bf16, tag=f"va_{s}", name=f"va_{s}")
                 for s in range(ST)]

        xmst = ExitStack()       # xmodT: dies after P2v
        xmodTp = pool(xmst, "xmodTp", 1)
        xmodT = [xmodTp.tile([P, S], bf16, tag=f"xm_{f}", name=f"xm_{f}") for f in range(FT)]

        # ---- P1: LN1 + modulate + transpose ----
        ph1 = ExitStack()
        xp = pool(ph1, "xp", 3)
        tp1 = pool(ph1, "tp1", 4, space="PSUM")
        for i in range(ST):
            x_sb = xp.tile([P, H], f32, tag="x")
            nc.sync.dma_start(out=x_sb[:, :], in_=x_in[i * P:(i + 1) * P, :])
            mv = smallp.tile([P, 2], f32, tag="mv")
            stt_all = smallp.tile([P, 2, 6], f32, tag="bnsta")
            nc.vector.bn_stats(out=stt_all[:, 0, :], in_=x_sb[:, 0:512])
            nc.vector.bn_stats(out=stt_all[:, 1, :], in_=x_sb[:, 512:1024])
            nc.vector.bn_aggr(out=mv[:, :], in_=stt_all[:, :, :])
            rstd = smallp.tile([P, 1], f32, tag="rstd")
            nc.scalar.activation(out=rstd[:, :], in_=mv[:, 1:2],
                                 func=Act.Sqrt, bias=eps_t[:, :])
            nc.vector.reciprocal(out=rstd[:, :], in_=rstd[:, :])
            xc = xp.tile([P, H], bf16, tag="xc")
            nc.vector.tensor_scalar(out=xc[:, :], in0=x_sb[:, :],
                                    scalar1=mv[:, 0:1], scalar2=rstd[:, :],
                                    op0=Alu.subtract, op1=Alu.mult)
            for f in range(FT):
                tps = tp1.tile([P, P], bf16, tag="tp")
                nc.tensor.transpose(tps[:, :], xc[:, f * P:(f + 1) * P],
                                    ident_b[:, :])
                nc.vector.tensor_scalar(
                    out=xmodT[f][:, i * P:(i + 1) * P], in0=tps[:, :],
                    scalar1=cm_sb[:, 8 + f:9 + f],
                    scalar2=cm_sb[:, f:f + 1],
                    op0=Alu.mult, op1=Alu.add)
        ph1.close()

        # ---- P2: kq^T (weights stationary) ----
        ph2 = ExitStack()
        wqkvp = pool(ph2, "wqkvp", 6)
        kqpsp = pool(ph2, "kqps", 4, space="PSUM")
        for m in range(16):
            ps = [kqpsp.tile([P, 512], f32, tag="kqps", name="kqps") for _ in range(2)]
            for kc in range(FT):
                wb = wqkvp.tile([P, P], bf16, tag="wqkvb")
                nc.sync.dma_start(out=wb[:, :],
                                  in_=wqkv_in[kc * P:(kc + 1) * P,
                                              m * P:(m + 1) * P])
                for hf in range(2):
                    nc.tensor.matmul(
                        ps[hf][:, :], lhsT=wb[:, :],
                        rhs=xmodT[kc][:, hf * 512:(hf + 1) * 512],
                        start=(kc == 0), stop=(kc == FT - 1))
            for hf in range(2):
                nc.scalar.copy(out=kqT[m][:, hf * 512:(hf + 1) * 512],
                               in_=ps[hf][:, :])
        ph2.close()

        # ---- P2v: v token-major + ones columns ----
        for s_ in range(ST):
            nc.vector.memset(v_aug[s_][:, :, 64:65], 1.0)
        ph2v = ExitStack()
        wvp = pool(ph2v, "wvp", 4)
        vpsp = pool(ph2v, "vps", 8, space="PSUM")
        for grp in (range(0, 4), range(4, 8)):
            vps = {(s_, hf): vpsp.tile([P, 512], f32, tag="vps", name="vps")
                   for s_ in grp for hf in range(2)}
            for kc in range(FT):
                wvb = [wvp.tile([P, 512], bf16, tag="wvb", name="wvb") for _ in range(2)]
                for hf in range(2):
                    nc.sync.dma_start(
                        out=wvb[hf][:, :],
                        in_=wqkv_in[kc * P:(kc + 1) * P,
                                    2048 + hf * 512:2048 + (hf + 1) * 512])
                for s_ in grp:
                    for hf in range(2):
                        nc.tensor.matmul(
                            vps[(s_, hf)][:, :],
                            lhsT=xmodT[kc][:, s_ * P:(s_ + 1) * P],
                            rhs=wvb[hf][:, :],
                            start=(kc == 0), stop=(kc == FT - 1))
            for s_ in grp:
                for hf in range(2):
                    nc.vector.tensor_copy(
                        out=v_aug[s_][:, hf * 8:(hf + 1) * 8, 0:64],
                        in_=vps[(s_, hf)][:, :].rearrange(
                            "p (a b) -> p a b", b=64))
        ph2v.close()
        xmst.close()

        # ---- P3: attention per head (incl. max-attn stat) ----
        ph3 = ExitStack()
        scpsp = pool(ph3, "scps", 2, space="PSUM")
        avpsp = pool(ph3, "avps", 3, space="PSUM")
        for h in range(NH):
            tkq = h // 2
            base = (h % 2) * 64
            m8h = m8p.tile([P, S], bf16, tag="m8", name=f"m8_{h}")
            pT = [pTp.tile([P, S], bf16, tag="pT", name="pT") for _ in range(ST)]
            for kt in range(ST):
                scp = scpsp.tile([P, S], f32, tag="scps")
                for hf in range(2):
                    nc.tensor.matmul(
                        scp[:, hf * 512:(hf + 1) * 512],
                        lhsT=kqT[tkq][base:base + 64, kt * P:(kt + 1) * P],
                        rhs=kqT[8 + tkq][base:base + 64,
                                         hf * 512:(hf + 1) * 512],
                        start=True, stop=True)
                nc.scalar.activation(out=pT[kt][:, :], in_=scp[:, :],
                                     func=Act.Exp, scale=0.125)
                if kt == 0:
                    nc.vector.tensor_copy(out=m8h[:, :], in_=pT[0][:, :])
                else:
                    nc.vector.tensor_tensor(out=m8h[:, :], in0=m8h[:, :],
                                            in1=pT[kt][:, :], op=Alu.max)
            for j in range(ST):
                yp = avpsp.tile([P, 65], f32, tag="avps", name="avps")
                for kt in range(ST):
                    nc.tensor.matmul(
                        yp[:, :], lhsT=pT[kt][:, j * P:(j + 1) * P],
                        rhs=v_aug[kt][:, h, :],
                        start=(kt == 0), stop=(kt == ST - 1))
                recD = smallp.tile([P, 1], f32, tag="recD")
                nc.vector.reciprocal(out=recD[:, :], in_=yp[:, 64:65])
                nc.vector.tensor_copy(out=st_recd[:, h * 8 + j:h * 8 + j + 1],
                                      in_=recD[:, :])
                nc.vector.tensor_scalar(
                    out=y_sb[j][:, h * 64:(h + 1) * 64], in0=yp[:, 0:64],
                    scalar1=recD[:, :], scalar2=None, op0=Alu.mult)
            for j in range(ST):
                tps = scpsp.tile([P, P], bf16, tag="scps", name="mx")
                nc.tensor.transpose(tps[:, :], m8h[:, j * P:(j + 1) * P],
                                    ident_b[:, :])
                nc.vector.reduce_max(out=st_maxp[:, h * 8 + j:h * 8 + j + 1],
                                     in_=tps[:, :], axis=mybir.AxisListType.X)
        ph3.close()
        ph3t = ExitStack()
        ytps = pool(ph3t, "ytps", 4, space="PSUM")
        for j in range(ST):
            for t in range(FT):
                tps = ytps.tile([P, P], bf16, tag="yt", name="yt")
                nc.tensor.transpose(tps[:, :], y_sb[j][:, t * P:(t + 1) * P],
                                    ident_b[:, :])
                nc.scalar.copy(out=yT_sb[t][:, j * P:(j + 1) * P],
                               in_=tps[:, :])
        ph3t.close()
        att.close()

        # ---- P4: attention out + gate + residual ----
        ph4 = ExitStack()
        watp = pool(ph4, "watp", 4)
        x2p = pool(ph4, "x2p", 2)
        xatp = pool(ph4, "xat", 3)
        aopsp = pool(ph4, "aops", 8, space="PSUM")
        for grp in (range(0, 4), range(4, 8)):
            aop = {(j, hf): aopsp.tile([P, 512], f32, tag="aops", name="aops")
                   for j in grp for hf in range(2)}
            for cc in range(FT):
                watb = [watp.tile([P, 512], bf16, tag="watb", name="watb")
                        for _ in range(2)]
                for hf in range(2):
                    nc.sync.dma_start(
                        out=watb[hf][:, :],
                        in_=wat_in[cc * P:(cc + 1) * P,
                                   hf * 512:(hf + 1) * 512])
                for j in grp:
                    for hf in range(2):
                        nc.tensor.matmul(
                            aop[(j, hf)][:, :],
                            lhsT=yT_sb[cc][:, j * P:(j + 1) * P],
                            rhs=watb[hf][:, :],
                            start=(cc == 0), stop=(cc == FT - 1))
            for j in grp:
                x_sb2 = x2p.tile([P, H], f32, tag="x2")
                nc.sync.dma_start(out=x_sb2[:, :],
                                  in_=x_in[j * P:(j + 1) * P, :])
                xat = xatp.tile([P, H], f32, tag="xat")
                for hf in range(2):
                    sl = slice(hf * 512, (hf + 1) * 512)
                    nc.vector.tensor_tensor(out=xat[:, sl],
                                            in0=aop[(j, hf)][:, :],
                                            in1=gate_msa_b[:, sl],
                                            op=Alu.mult)
                jt = junkp.tile([P, H], bf16, tag="jk")
                nc.scalar.activation(out=jt[:, :], in_=xat[:, :],
                                     func=Act.Square,
                                     accum_out=st_ssqa[:, j:j + 1])
                nc.vector.tensor_tensor(out=x1[j][:, :], in0=xat[:, :],
                                        in1=x_sb2[:, :], op=Alu.add)
        ph4.close()
        yst.close()

        # ---- P5: LN2 + modulate + transpose ----
        mid = ExitStack()        # x2T: dies after P6
        x2Tp = pool(mid, "x2Tp", 1)
        x2T = [x2Tp.tile([P, S], bf16, tag=f"x2T_{f}", name=f"x2T_{f}") for f in range(FT)]
        ph5 = ExitStack()
        xc2p = pool(ph5, "xc2p", 3)
        tp2 = pool(ph5, "tp2", 4, space="PSUM")
        for j in range(ST):
            stt_all = smallp.tile([P, 2, 6], f32, tag="bnsta")
            mv = smallp.tile([P, 2], f32, tag="mv")
            nc.vector.bn_stats(out=stt_all[:, 0, :], in_=x1[j][:, 0:512])
            nc.vector.bn_stats(out=stt_all[:, 1, :], in_=x1[j][:, 512:1024])
            nc.vector.bn_aggr(out=mv[:, :], in_=stt_all[:, :, :])
            nc.vector.tensor_copy(out=st_ln2[:, j:j + 1], in_=mv[:, 0:1])
            nc.vector.tensor_copy(out=st_ln2[:, 8 + j:9 + j], in_=mv[:, 1:2])
            rstd = smallp.tile([P, 1], f32, tag="rstd")
            nc.scalar.activation(out=rstd[:, :], in_=mv[:, 1:2],
                                 func=Act.Sqrt, bias=eps_t[:, :])
            nc.vector.reciprocal(out=rstd[:, :], in_=rstd[:, :])
            xc2 = xc2p.tile([P, H], bf16, tag="xc2")
            nc.vector.tensor_scalar(out=xc2[:, :], in0=x1[j][:, :],
                                    scalar1=mv[:, 0:1], scalar2=rstd[:, :],
                                    op0=Alu.subtract, op1=Alu.mult)
            for f in range(FT):
                tps = tp2.tile([P, P], bf16, tag="tp2")
                nc.tensor.transpose(tps[:, :], xc2[:, f * P:(f + 1) * P],
                                    ident_b[:, :])
                nc.vector.tensor_scalar(
                    out=x2T[f][:, j * P:(j + 1) * P], in0=tps[:, :],
                    scalar1=cm_sb[:, 32 + f:33 + f],
                    scalar2=cm_sb[:, 24 + f:25 + f],
                    op0=Alu.mult, op1=Alu.add)
        ph5.close()

        # ---- P6: MLP1 + gelu + stats ----
        gelust = ExitStack()     # geluT: dies after P7
        gelup = pool(gelust, "gelup", 1)
        geluT = [gelup.tile([P, S], bf16, tag=f"g_{m}", name=f"g_{m}") for m in range(MT)]
        ph6 = ExitStack()
        wm1p = pool(ph6, "wm1p", 6)
        m1psp = pool(ph6, "m1ps", 4, space="PSUM")
        for m in range(MT):
            ps = [m1psp.tile([P, 512], f32, tag="m1ps", name="m1ps") for _ in range(2)]
            for kc in range(FT):
                wb = wm1p.tile([P, P], bf16, tag="wm1b")
                nc.sync.dma_start(out=wb[:, :],
                                  in_=wm1_in[kc * P:(kc + 1) * P,
                                             m * P:(m + 1) * P])
                for hf in range(2):
                    nc.tensor.matmul(
                        ps[hf][:, :], lhsT=wb[:, :],
                        rhs=x2T[kc][:, hf * 512:(hf + 1) * 512],
                        start=(kc == 0), stop=(kc == FT - 1))
            for hf in range(2):
                nc.scalar.activation(
                    out=geluT[m][:, hf * 512:(hf + 1) * 512],
                    in_=ps[hf][:, :], func=Act.Gelu_apprx_tanh)
                jt = junkp.tile([P, 512], bf16, tag="jk2")
                nc.scalar.activation(
                    out=jt[:, :], in_=ps[hf][:, :], func=Act.Square,
                    accum_out=st_ssqh[:, m * 2 + hf:m * 2 + hf + 1])
                jt2 = junkp.tile([P, 512], bf16, tag="jk3")
                nc.vector.tensor_scalar(
                    out=jt2[:, :], in0=ps[hf][:, :],
                    scalar1=0.0, scalar2=0.0,
                    op0=Alu.is_gt, op1=Alu.add,
                    accum_out=st_cnt[:, m * 2 + hf:m * 2 + hf + 1])
        ph6.close()

        # ---- P7: MLP2 + gate + residual ----
        ph7 = ExitStack()
        wm2p = pool(ph7, "wm2p", 3)
        xmtp = pool(ph7, "xmt", 3)
        xotp = pool(ph7, "xot", 3)
        m2psp = pool(ph7, "m2ps", 8, space="PSUM")
        for grp in (range(0, 4), range(4, 8)):
            mps = {(j, hf): m2psp.tile([P, 512], f32, tag="m2ps", name="m2ps")
                   for j in grp for hf in range(2)}
            for mc in range(MT):
                wb = wm2p.tile([P, H], bf16, tag="wm2b")
                nc.sync.dma_start(out=wb[:, :],
                                  in_=wm2_in[mc * P:(mc + 1) * P, :])
                for j in grp:
                    for hf in range(2):
                        nc.tensor.matmul(
                            mps[(j, hf)][:, :],
                            lhsT=geluT[mc][:, j * P:(j + 1) * P],
                            rhs=wb[:, hf * 512:(hf + 1) * 512],
                            start=(mc == 0), stop=(mc == MT - 1))
            for j in grp:
                xmt = xmtp.tile([P, H], f32, tag="xmt")
                for hf in range(2):
                    sl = slice(hf * 512, (hf + 1) * 512)
                    nc.vector.tensor_tensor(out=xmt[:, sl],
                                            in0=mps[(j, hf)][:, :],
                                            in1=gate_mlp_b[:, sl],
                                            op=Alu.mult)
                jt = junkp.tile([P, H], bf16, tag="jk")
                nc.scalar.activation(out=jt[:, :], in_=xmt[:, :],
                                     func=Act.Square,
                                     accum_out=st_ssqm[:, j:j + 1])
                xo = xotp.tile([P, H], f32, tag="xo")
                nc.vector.tensor_tensor(out=xo[:, :], in0=xmt[:, :],
                                        in1=x1[j][:, :], op=Alu.add)
                nc.sync.dma_start(out=xout[j * P:(j + 1) * P, :],
                                  in_=xo[:, :])
        ph7.close()
        gelust.close()
        mid.close()
        late.close()

        # ---- stats out ----
        nc.sync.dma_start(out=st_out[:, 0:64], in_=st_cnt[:, :])
        nc.sync.dma_start(out=st_out[:, 64:128], in_=st_ssqh[:, :])
        nc.sync.dma_start(out=st_out[:, 128:136], in_=st_ssqa[:, :])
        nc.sync.dma_start(out=st_out[:, 136:144], in_=st_ssqm[:, :])
        nc.sync.dma_start(out=st_out[:, 144:160], in_=st_ln2[:, :])
        nc.sync.dma_start(out=st_out[:, 160:288], in_=st_maxp[:, :])
        nc.sync.dma_start(out=recd_out[:, :], in_=st_recd[:, :])
        root.close()

    nc.compile()
    return nc


def get_nc():
    if "nc" not in _CACHE:
        _CACHE["nc"] = _build_nc()
    return _CACHE["nc"]


def _prep_in_maps(x, c, w_cond, w_qkv, w_attn_out, w_mlp1, w_mlp2):
    bf = ml_dtypes.bfloat16
    wc = np.ascontiguousarray(np.asarray(w_cond, dtype=np.float32)).astype(bf)
    wqkv = np.ascontiguousarray(np.asarray(w_qkv, dtype=np.float32)).astype(bf)
    wat = np.ascontiguousarray(np.asarray(w_attn_out, dtype=np.float32)).astype(bf)
    wm1 = np.ascontiguousarray(np.asarray(w_mlp1, dtype=np.float32)).astype(bf)
    wm2 = np.ascontiguousarray(np.asarray(w_mlp2, dtype=np.float32)).astype(bf)
    x = np.asarray(x, dtype=np.float32)
    c = np.asarray(c, dtype=np.float32)
    return [{
        "x": np.ascontiguousarray(x[b]),
        "cvec": np.ascontiguousarray(c[b]),
        "wc": wc, "wqkv": wqkv, "wat": wat, "wm1": wm1, "wm2": wm2,
    } for b in range(NCORES)]


def _host_finish(x, results):
    x = np.asarray(x, dtype=np.float32)
    xout = np.stack([results[b]["xout"] for b in range(NCORES)], axis=0)

    cnt = np.zeros((P, 32), np.float64)
    ssqh = 0.0
    ssqa = 0.0
    ssqm = 0.0
    ln2_sum = 0.0
    maxw_sum = 0.0
    for b in range(NCORES):
        st = np.asarray(results[b]["stats"], dtype=np.float64)
        recd = np.asarray(results[b]["recd"], dtype=np.float64)
        cnt += st[:, 0:64].reshape(P, 32, 2).sum(axis=2)
        ssqh += st[:, 64:128].sum()
        ssqa += st[:, 128:136].sum()
        ssqm += st[:, 136:144].sum()
        mean = st[:, 144:152]
        var = st[:, 152:160]
        ln2_sum += (var + mean ** 2).sum()
        maxp = st[:, 160:288]          # [p, h*8+j]
        maxw_sum += (maxp * recd).sum()

    frac = cnt / (B * S)               # [p, m] unit = m*128+p
    info_relu_diff = np.abs(frac - 0.5).mean()
    info_relu_zero = (cnt == 0).mean()
    info_relu_positive = (cnt == B * S).mean()
    info_relu_norm = np.sqrt(ssqh / (B * S * MLP))
    info_max_attn = maxw_sum / (B * NH * S)
    rms_x = np.sqrt((x.astype(np.float64) ** 2).mean())
    info_attn_ratio = np.sqrt(ssqa / (B * S * H)) / rms_x
    rms_x1 = np.sqrt(ln2_sum / (B * S))
    info_mlp_ratio = np.sqrt(ssqm / (B * S * H)) / rms_x1

    infos = (np.float32(info_relu_diff), np.float32(info_relu_zero),
             np.float32(info_relu_positive), np.float32(info_max_attn),
             np.float32(info_attn_ratio), np.float32(info_mlp_ratio),
             np.float32(info_relu_norm))
    return xout, infos


def kernel(x, c, w_cond, w_qkv, w_attn_out, w_mlp1, w_mlp2):
    from concourse.bass_utils import run_bass_kernel_spmd
    nc = get_nc()
    in_maps = _prep_in_maps(x, c, w_cond, w_qkv, w_attn_out, w_mlp1, w_mlp2)
    res = run_bass_kernel_spmd(nc, in_maps, list(range(NCORES)))
    _CACHE["last_res"] = res
    return _host_finish(x, res.results)
